# revision 9
# baseline (speedup 1.0000x reference)
"""Trainium2 Bass kernel for nn_Loss_64295660421321 (NT-Xent contrastive + graph loss).

Math (reference):
  contrastive = sum_i [logsumexp_{j != i} sim_ij - pos_i] / N,  sim = (h h^T)/T, h=[h_i;h_j]
  graph       = (t1 + t2 - 2*t3) / (M*K)
    t1 = sum_m rowsum(G)_m * ||sub_x_m||^2
    t2 = sum_k colsum(G)_k * ||all_x_k||^2
    t3 = sum((G @ all_x) * sub_x)

Device decomposition (8 cores, SPMD, all core differences encoded in input data):
  * g = sqrt(2)*[h_i; h_j]  ->  sim = g g^T (the 1/T=2 scale folded into g).
    Core c owns sim rows [1024c, 1024c+1024). gT sent per-core with columns
    rolled by -1024c so the diagonal block lands at a core-invariant position.
    Row-wise: exp(sim_ij - mhat_i) with a per-row shift mhat (host-computed
    norm-based bound, validated to sit in fp32 exp range), diag masked to -1e30
    pre-exp; ScalarE activation(Exp, bias, accum_out) fuses exp + row-sum.
    lse_i = mhat_i + log(sum).  pos is O(N*D) and computed on host.
  * graph: P = G^T @ [sub_x | 1 | s]  ([K,130], s_m=||sub_x_m||^2), then
    t1+t2-2*t3 = sum(P * C) with C = [-2*all_x | q | 1], q_k=||all_x_k||^2.
    Core c owns G rows [512c, 512c+512): P partials accumulate in PSUM over
    the core's 4 row-chunks; DVE tensor_tensor_reduce does the (P*C) dot.
Host combines per-core scalars in float64.
"""

import numpy as np

import concourse.bass as bass
import concourse.tile as tile
from concourse import mybir
from concourse.bass_utils import run_bass_kernel_spmd

F32 = mybir.dt.float32
BF16 = mybir.dt.bfloat16
U32 = mybir.dt.uint32
LN2 = float(np.log(2.0))

NCORES = 8
B, D, K = 4096, 128, 8192
N = 2 * B                     # 8192 sim rows/cols
RPC = N // NCORES             # 1024 sim rows per core
NRB = RPC // 128              # 8 row-blocks of 128
MPC = B // NCORES             # 512 G rows per core
NMC = MPC // 128              # 4 m-chunks
NKB = K // 128                # 64 k-blocks
NKG = NKB // 4                # 16 k-groups (4 k-blocks each)
JA = D + 2                    # 130 augmented columns

MASK_NEG = np.float32(-1.0e30)

LAST_RESULTS = None           # test harness introspection


def _split_excess_waits(nc, max_waits=1):
    """This walrus build rejects >1 semaphore wait per instruction; peel extras
    onto preceding NoOps on the same engine (same-engine order makes it safe)."""
    for fn in nc.m.functions:
        for blk in fn.blocks:
            new_insts = []
            for inst in blk.instructions:
                si = getattr(inst, "sync_info", None)
                if si is not None and si.on_wait is not None and len(si.on_wait) > max_waits:
                    waits = list(si.on_wait)
                    while len(waits) > max_waits:
                        head, waits = waits[:max_waits], waits[max_waits:]
                        new_insts.append(
                            mybir.InstNoOp(
                                name=nc.get_next_instruction_name(),
                                engine=inst.engine,
                                ins=[],
                                outs=[],
                                sync_info=mybir.SyncInfo(on_wait=head, on_update=[]),
                                text_hint="wait_split",
                            )
                        )
                    si.on_wait = waits
                new_insts.append(inst)
            blk.instructions = new_insts


def _build():
    nc = bass.Bass()
    gT_d = nc.dram_tensor("gT", [128, N], F32, kind="ExternalInput")
    G_d = nc.dram_tensor("G", [MPC, K], F32, kind="ExternalInput")
    C_d = nc.dram_tensor("C", [128, NKB * JA], F32, kind="ExternalInput")
    B_d = nc.dram_tensor("Baug", [128, NMC, JA], F32, kind="ExternalInput")
    bias_d = nc.dram_tensor("bias", [128, 2 * NRB], F32, kind="ExternalInput")
    mask_d = nc.dram_tensor("mask", [128, 128], F32, kind="ExternalInput")
    out_d = nc.dram_tensor("out", [128, 2], F32, kind="ExternalOutput")

    with tile.TileContext(nc) as tc:
        with (
            tc.tile_pool(name="singles", bufs=1) as singles,
            tc.tile_pool(name="gpool", bufs=3) as gpool,
            tc.tile_pool(name="ps", bufs=2, space="PSUM") as ps,
        ):
            gT = singles.tile([128, N], F32)
            C = singles.tile([128, NKB * JA], F32)
            Ba = singles.tile([128, NMC, JA], F32)
            bias = singles.tile([128, 2 * NRB], F32)
            mask = singles.tile([128, 128], F32)
            SUME = singles.tile([128, 4 * NRB], F32)
            ACC = singles.tile([128, NKB], F32)
            Esc = singles.tile([128, 2048], BF16)
            Tsc = singles.tile([128, 4, JA], F32)
            Stmp = singles.tile([128, NRB], F32)
            EU = singles.tile([128, NRB], U32)
            EF = singles.tile([128, NRB], F32)
            MU = singles.tile([128, NRB], U32)
            Ltmp = singles.tile([128, NRB], F32)
            L2 = singles.tile([128, NRB], F32)
            OUT = singles.tile([128, 2], F32)

            nc.sync.dma_start(out=gT, in_=gT_d[:, :])
            nc.sync.dma_start(out=C, in_=C_d[:, :])
            nc.sync.dma_start(out=Ba, in_=B_d[:, :, :])
            nc.sync.dma_start(out=bias, in_=bias_d[:, :])
            nc.sync.dma_start(out=mask, in_=mask_d[:, :])

            G_re = G_d[:, :].rearrange("(mc p) k -> p mc k", p=128)

            def emit_sim_half(rb, h):
                # One [128, 2048] PSUM tile = sim cols [2048h, 2048h+2048) of row-block rb.
                slot = ps.tile([128, 2048], F32, tag="ps")
                for qi in range(4):
                    ct = 4 * h + qi
                    nc.tensor.matmul(
                        slot[:, 512 * qi : 512 * qi + 512],
                        lhsT=gT[:, 128 * rb : 128 * rb + 128],
                        rhs=gT[:, 512 * ct : 512 * ct + 512],
                        start=True,
                        stop=True,
                    )
                if h == 0:
                    # rolled diag block sits at cols [128rb, 128rb+128) (always in h==0)
                    dslice = slot[:, 128 * rb : 128 * rb + 128]
                    nc.vector.tensor_add(dslice, dslice, mask)
                nc.scalar.activation(
                    out=Esc,
                    in_=slot,
                    func=mybir.ActivationFunctionType.Exp,
                    bias=bias[:, rb : rb + 1],
                    scale=1.0,
                    accum_out=SUME[:, 4 * rb + h : 4 * rb + h + 1],
                )

            def emit_graph_group(g):
                Gg = gpool.tile([128, NMC, 512], F32, tag="gg")
                nc.sync.dma_start(out=Gg, in_=G_re[:, :, 512 * g : 512 * g + 512])
                slot = ps.tile([128, 2048], F32, tag="ps")
                for kb in range(4):
                    for mc in range(NMC):
                        nc.tensor.matmul(
                            slot[:, 512 * kb : 512 * kb + JA],
                            lhsT=Gg[:, mc, 128 * kb : 128 * kb + 128],
                            rhs=Ba[:, mc, :],
                            start=(mc == 0),
                            stop=(mc == NMC - 1),
                        )
                for kb in range(4):
                    nc.vector.scalar_tensor_tensor(
                        out=Tsc[:, kb, :],
                        in0=slot[:, 512 * kb : 512 * kb + JA],
                        scalar=1.0,
                        in1=C[:, JA * (4 * g + kb) : JA * (4 * g + kb + 1)],
                        op0=mybir.AluOpType.mult,
                        op1=mybir.AluOpType.mult,
                        accum_out=ACC[:, 4 * g + kb : 4 * g + kb + 1],
                    )

            for rb in range(NRB):
                emit_sim_half(rb, 0)
                emit_sim_half(rb, 1)
                emit_graph_group(2 * rb)
                emit_sim_half(rb, 2)
                emit_sim_half(rb, 3)
                emit_graph_group(2 * rb + 1)

            # lse finish: S = sum over the 4 col-chunks, lse = log(S) + mhat.
            # ACT Ln is only valid on ~[2^-64, 2^64] and S reaches e^65, so do a
            # frexp split: log(S) = Ln(mantissa) + exponent*ln2 - 127*ln2 (the
            # -127*ln2 is folded into the host-side bias columns).
            nc.vector.reduce_sum(
                out=Stmp,
                in_=SUME.rearrange("p (r i) -> p r i", i=4),
                axis=mybir.AxisListType.X,
            )
            su = Stmp.bitcast(U32)
            nc.vector.tensor_scalar(
                out=EU,
                in0=su,
                scalar1=23,
                scalar2=None,
                op0=mybir.AluOpType.logical_shift_right,
            )
            nc.vector.tensor_copy(EF, EU)  # uint -> float value cast
            nc.vector.tensor_scalar(
                out=MU,
                in0=su,
                scalar1=0x007FFFFF,
                scalar2=0x3F800000,
                op0=mybir.AluOpType.bitwise_and,
                op1=mybir.AluOpType.bitwise_or,
            )
            nc.scalar.activation(
                out=Ltmp, in_=MU.bitcast(F32), func=mybir.ActivationFunctionType.Ln
            )
            nc.vector.scalar_tensor_tensor(
                out=L2,
                in0=EF,
                scalar=LN2,
                in1=Ltmp,
                op0=mybir.AluOpType.mult,
                op1=mybir.AluOpType.add,
            )
            nc.vector.tensor_add(L2, L2, bias[:, NRB : 2 * NRB])
            nc.vector.reduce_sum(out=OUT[:, 0:1], in_=L2, axis=mybir.AxisListType.X)
            nc.vector.reduce_sum(out=OUT[:, 1:2], in_=ACC, axis=mybir.AxisListType.X)
            nc.sync.dma_start(out=out_d[:, :], in_=OUT)

    _split_excess_waits(nc)
    return nc


_NC_CACHE = []


def _get_nc():
    if not _NC_CACHE:
        _NC_CACHE.append(_build())
    return _NC_CACHE[0]


def kernel(h_i, h_j, sub_graph, sub_x, all_x):
    h_i = np.asarray(h_i, dtype=np.float32)
    h_j = np.asarray(h_j, dtype=np.float32)
    G = np.asarray(sub_graph, dtype=np.float32)
    sub_x = np.asarray(sub_x, dtype=np.float32)
    all_x = np.asarray(all_x, dtype=np.float32)

    g = np.concatenate([h_i, h_j], axis=0) * np.float32(np.sqrt(2.0))  # [N, D]
    gT = np.ascontiguousarray(g.T)  # [D=128, N]

    norms = np.sqrt(np.sum(g.astype(np.float64) ** 2, axis=1))  # [N]
    mhat = (6.005 * norms + 12.0).astype(np.float32)  # row shifts

    s = np.sum(sub_x.astype(np.float64) ** 2, axis=1).astype(np.float32)  # [B]
    q = np.sum(all_x.astype(np.float64) ** 2, axis=1).astype(np.float32)  # [K]
    Baug = np.concatenate(
        [sub_x, np.ones((B, 1), np.float32), s[:, None]], axis=1
    )  # [B, 130]
    Cfull = np.concatenate(
        [-2.0 * all_x, q[:, None], np.ones((K, 1), np.float32)], axis=1
    )  # [K, 130]
    # pack C so k-block kb, row p, col j lives at C_packed[p, kb*130 + j]
    C_packed = np.ascontiguousarray(
        Cfull.reshape(NKB, 128, JA).transpose(1, 0, 2).reshape(128, NKB * JA)
    )

    mask = np.zeros((128, 128), np.float32)
    np.fill_diagonal(mask, MASK_NEG)

    in_maps = []
    for c in range(NCORES):
        gT_c = np.ascontiguousarray(np.roll(gT, -RPC * c, axis=1))
        rows = np.arange(RPC * c, RPC * (c + 1))
        mh = mhat[rows].reshape(NRB, 128).T  # [128, 8]; col rb = shift for that block
        bias_c = np.concatenate([-mh, mh - np.float32(127.0 * LN2)], axis=1).astype(
            np.float32
        )  # [128, 16]: exp bias | lse add-back (frexp's -127*ln2 folded in)
        G_c = np.ascontiguousarray(G[MPC * c : MPC * (c + 1), :])
        mrows = np.arange(MPC * c, MPC * (c + 1))
        B_c = np.ascontiguousarray(
            Baug[mrows].reshape(NMC, 128, JA).transpose(1, 0, 2)
        )  # [128, 4, 130]
        in_maps.append(
            {
                "gT": gT_c,
                "G": G_c,
                "C": C_packed,
                "Baug": B_c,
                "bias": bias_c,
                "mask": mask,
            }
        )

    import os

    trace = bool(int(os.environ.get("BASS_KERNEL_TRACE", "0")))
    res = run_bass_kernel_spmd(
        _get_nc(), in_maps, core_ids=list(range(NCORES)), trace=trace
    )
    global LAST_RESULTS
    LAST_RESULTS = res

    lse_sum = 0.0
    graph_sum = 0.0
    for c in range(NCORES):
        o = res.results[c]["out"].astype(np.float64)
        lse_sum += float(o[:, 0].sum())
        graph_sum += float(o[:, 1].sum())

    pos_total = 4.0 * float(
        np.sum(h_i.astype(np.float64) * h_j.astype(np.float64))
    )
    loss = (lse_sum - pos_total) / N + graph_sum / (float(B) * float(K))
    return np.float32(loss)


# revision 11
# speedup vs baseline: 1.5654x; 1.5654x over previous
"""Trainium2 Bass kernel for nn_Loss_64295660421321 (NT-Xent contrastive + graph loss).

Math (reference):
  contrastive = sum_i [logsumexp_{j != i} sim_ij - pos_i] / N,  sim = (h h^T)/T, h=[h_i;h_j]
  graph       = (t1 + t2 - 2*t3) / (M*K)
    t1 = sum_m rowsum(G)_m * ||sub_x_m||^2
    t2 = sum_k colsum(G)_k * ||all_x_k||^2
    t3 = sum((G @ all_x) * sub_x)

Device decomposition (8 cores, SPMD, all core differences encoded in input data):
  * g = sqrt(2)*[h_i; h_j]  ->  sim = g g^T (the 1/T=2 scale folded into g).
    Core c owns sim rows [1024c, 1024c+1024). gT sent per-core with columns
    rolled by -1024c so the diagonal block lands at a core-invariant position.
    Row-wise: exp(sim_ij - mhat_i) with a per-row shift mhat (host-computed
    norm-based bound, validated to sit in fp32 exp range), diag masked to -1e30
    pre-exp; ScalarE activation(Exp, bias, accum_out) fuses exp + row-sum.
    lse_i = mhat_i + log(sum).  pos is O(N*D) and computed on host.
  * graph: P = G^T @ [sub_x | 1 | s]  ([K,130], s_m=||sub_x_m||^2), then
    t1+t2-2*t3 = sum(P * C) with C = [-2*all_x | q | 1], q_k=||all_x_k||^2.
    Core c owns G rows [512c, 512c+512): P partials accumulate in PSUM over
    the core's 4 row-chunks; DVE tensor_tensor_reduce does the (P*C) dot.
Host combines per-core scalars in float64.
"""

import numpy as np

import concourse.bass as bass
import concourse.tile as tile
from concourse import mybir
from concourse.bass_utils import run_bass_kernel_spmd

F32 = mybir.dt.float32
BF16 = mybir.dt.bfloat16
U32 = mybir.dt.uint32
F16 = mybir.dt.float16
LN2 = float(np.log(2.0))

NCORES = 8
B, D, K = 4096, 128, 8192
N = 2 * B                     # 8192 sim rows/cols
RPC = N // NCORES             # 1024 sim rows per core
NRB = RPC // 128              # 8 row-blocks of 128
MPC = B // NCORES             # 512 G rows per core
NMC = MPC // 128              # 4 m-chunks
NKB = K // 128                # 64 k-blocks
NKG = NKB // 4                # 16 k-groups (4 k-blocks each)
JA = D + 2                    # 130 augmented columns

MASK_NEG = np.float32(-1.0e30)

LAST_RESULTS = None           # test harness introspection


def _split_excess_waits(nc, max_waits=1):
    """This walrus build rejects >1 semaphore wait per instruction; peel extras
    onto preceding NoOps on the same engine (same-engine order makes it safe)."""
    for fn in nc.m.functions:
        for blk in fn.blocks:
            new_insts = []
            for inst in blk.instructions:
                si = getattr(inst, "sync_info", None)
                if si is not None and si.on_wait is not None and len(si.on_wait) > max_waits:
                    waits = list(si.on_wait)
                    while len(waits) > max_waits:
                        head, waits = waits[:max_waits], waits[max_waits:]
                        new_insts.append(
                            mybir.InstNoOp(
                                name=nc.get_next_instruction_name(),
                                engine=inst.engine,
                                ins=[],
                                outs=[],
                                sync_info=mybir.SyncInfo(on_wait=head, on_update=[]),
                                text_hint="wait_split",
                            )
                        )
                    si.on_wait = waits
                new_insts.append(inst)
            blk.instructions = new_insts


def _build():
    nc = bass.Bass()
    gT_d = nc.dram_tensor("gT", [128, N], F16, kind="ExternalInput")
    G_d = nc.dram_tensor("G", [MPC, K], F16, kind="ExternalInput")
    C_d = nc.dram_tensor("C", [128, NKB * JA], F32, kind="ExternalInput")
    B_d = nc.dram_tensor("Baug", [128, NMC, JA], F16, kind="ExternalInput")
    bias_d = nc.dram_tensor("bias", [128, 2 * NRB], F32, kind="ExternalInput")
    mask_d = nc.dram_tensor("mask", [128, 128], F32, kind="ExternalInput")
    out_d = nc.dram_tensor("out", [128, 2], F32, kind="ExternalOutput")

    with tile.TileContext(nc) as tc:
        with (
            tc.tile_pool(name="singles", bufs=1) as singles,
            tc.tile_pool(name="gpool", bufs=3) as gpool,
            tc.tile_pool(name="ps", bufs=2, space="PSUM") as ps,
        ):
            gT = singles.tile([128, N], F16)
            C = singles.tile([128, NKB * JA], F32)
            Ba = singles.tile([128, NMC, JA], F16)
            bias = singles.tile([128, 2 * NRB], F32)
            mask = singles.tile([128, 128], F32)
            SUME = singles.tile([128, 4 * NRB], F32)
            ACC = singles.tile([128, NKB], F32)
            Esc = singles.tile([128, 2048], BF16)
            Tsc = singles.tile([128, 4, JA], F32)
            Stmp = singles.tile([128, NRB], F32)
            EU = singles.tile([128, NRB], U32)
            EF = singles.tile([128, NRB], F32)
            MU = singles.tile([128, NRB], U32)
            Ltmp = singles.tile([128, NRB], F32)
            L2 = singles.tile([128, NRB], F32)
            OUT = singles.tile([128, 2], F32)

            nc.sync.dma_start(out=gT, in_=gT_d[:, :])
            nc.sync.dma_start(out=C, in_=C_d[:, :])
            nc.sync.dma_start(out=Ba, in_=B_d[:, :, :])
            nc.sync.dma_start(out=bias, in_=bias_d[:, :])
            nc.sync.dma_start(out=mask, in_=mask_d[:, :])

            G_re = G_d[:, :].rearrange("(mc p) k -> p mc k", p=128)

            def emit_sim_half(rb, h):
                # One [128, 2048] PSUM tile = sim cols [2048h, 2048h+2048) of row-block rb.
                slot = ps.tile([128, 2048], F32, tag="ps")
                for qi in range(4):
                    ct = 4 * h + qi
                    nc.tensor.matmul(
                        slot[:, 512 * qi : 512 * qi + 512],
                        lhsT=gT[:, 128 * rb : 128 * rb + 128],
                        rhs=gT[:, 512 * ct : 512 * ct + 512],
                        start=True,
                        stop=True,
                    )
                if h == 0:
                    # rolled diag block sits at cols [128rb, 128rb+128) (always in h==0)
                    dslice = slot[:, 128 * rb : 128 * rb + 128]
                    nc.vector.tensor_add(dslice, dslice, mask)
                nc.scalar.activation(
                    out=Esc,
                    in_=slot,
                    func=mybir.ActivationFunctionType.Exp,
                    bias=bias[:, rb : rb + 1],
                    scale=1.0,
                    accum_out=SUME[:, 4 * rb + h : 4 * rb + h + 1],
                )

            def emit_graph_group(g):
                Gg = gpool.tile([128, NMC, 512], F16, tag="gg")
                nc.sync.dma_start(out=Gg, in_=G_re[:, :, 512 * g : 512 * g + 512])
                slot = ps.tile([128, 2048], F32, tag="ps")
                for kb in range(4):
                    for mc in range(NMC):
                        nc.tensor.matmul(
                            slot[:, 512 * kb : 512 * kb + JA],
                            lhsT=Gg[:, mc, 128 * kb : 128 * kb + 128],
                            rhs=Ba[:, mc, :],
                            start=(mc == 0),
                            stop=(mc == NMC - 1),
                        )
                for kb in range(4):
                    nc.vector.scalar_tensor_tensor(
                        out=Tsc[:, kb, :],
                        in0=slot[:, 512 * kb : 512 * kb + JA],
                        scalar=1.0,
                        in1=C[:, JA * (4 * g + kb) : JA * (4 * g + kb + 1)],
                        op0=mybir.AluOpType.mult,
                        op1=mybir.AluOpType.mult,
                        accum_out=ACC[:, 4 * g + kb : 4 * g + kb + 1],
                    )

            for rb in range(NRB):
                emit_sim_half(rb, 0)
                emit_sim_half(rb, 1)
                emit_graph_group(2 * rb)
                emit_sim_half(rb, 2)
                emit_sim_half(rb, 3)
                emit_graph_group(2 * rb + 1)

            # lse finish: S = sum over the 4 col-chunks, lse = log(S) + mhat.
            # ACT Ln is only valid on ~[2^-64, 2^64] and S reaches e^65, so do a
            # frexp split: log(S) = Ln(mantissa) + exponent*ln2 - 127*ln2 (the
            # -127*ln2 is folded into the host-side bias columns).
            nc.vector.reduce_sum(
                out=Stmp,
                in_=SUME.rearrange("p (r i) -> p r i", i=4),
                axis=mybir.AxisListType.X,
            )
            su = Stmp.bitcast(U32)
            nc.vector.tensor_scalar(
                out=EU,
                in0=su,
                scalar1=23,
                scalar2=None,
                op0=mybir.AluOpType.logical_shift_right,
            )
            nc.vector.tensor_copy(EF, EU)  # uint -> float value cast
            nc.vector.tensor_scalar(
                out=MU,
                in0=su,
                scalar1=0x007FFFFF,
                scalar2=0x3F800000,
                op0=mybir.AluOpType.bitwise_and,
                op1=mybir.AluOpType.bitwise_or,
            )
            nc.scalar.activation(
                out=Ltmp, in_=MU.bitcast(F32), func=mybir.ActivationFunctionType.Ln
            )
            nc.vector.scalar_tensor_tensor(
                out=L2,
                in0=EF,
                scalar=LN2,
                in1=Ltmp,
                op0=mybir.AluOpType.mult,
                op1=mybir.AluOpType.add,
            )
            nc.vector.tensor_add(L2, L2, bias[:, NRB : 2 * NRB])
            nc.vector.reduce_sum(out=OUT[:, 0:1], in_=L2, axis=mybir.AxisListType.X)
            nc.vector.reduce_sum(out=OUT[:, 1:2], in_=ACC, axis=mybir.AxisListType.X)
            nc.sync.dma_start(out=out_d[:, :], in_=OUT)

    _split_excess_waits(nc)
    return nc


_NC_CACHE = []


def _get_nc():
    if not _NC_CACHE:
        _NC_CACHE.append(_build())
    return _NC_CACHE[0]


def kernel(h_i, h_j, sub_graph, sub_x, all_x):
    h_i = np.asarray(h_i, dtype=np.float32)
    h_j = np.asarray(h_j, dtype=np.float32)
    G = np.asarray(sub_graph, dtype=np.float32)
    sub_x = np.asarray(sub_x, dtype=np.float32)
    all_x = np.asarray(all_x, dtype=np.float32)

    g = np.concatenate([h_i, h_j], axis=0) * np.float32(np.sqrt(2.0))  # [N, D]
    gT = np.ascontiguousarray(g.T.astype(np.float16))  # [D=128, N] fp16 for PE

    norms = np.sqrt(np.sum(g.astype(np.float64) ** 2, axis=1))  # [N]
    mhat = (6.005 * norms + 12.0).astype(np.float32)  # row shifts

    s = np.sum(sub_x.astype(np.float64) ** 2, axis=1).astype(np.float32)  # [B]
    q = np.sum(all_x.astype(np.float64) ** 2, axis=1).astype(np.float32)  # [K]
    Baug = np.concatenate(
        [sub_x, np.ones((B, 1), np.float32), s[:, None]], axis=1
    ).astype(np.float16)  # [B, 130] fp16 for PE
    Cfull = np.concatenate(
        [-2.0 * all_x, q[:, None], np.ones((K, 1), np.float32)], axis=1
    )  # [K, 130]
    # pack C so k-block kb, row p, col j lives at C_packed[p, kb*130 + j]
    C_packed = np.ascontiguousarray(
        Cfull.reshape(NKB, 128, JA).transpose(1, 0, 2).reshape(128, NKB * JA)
    )

    mask = np.zeros((128, 128), np.float32)
    np.fill_diagonal(mask, MASK_NEG)

    in_maps = []
    for c in range(NCORES):
        gT_c = np.ascontiguousarray(np.roll(gT, -RPC * c, axis=1))
        rows = np.arange(RPC * c, RPC * (c + 1))
        mh = mhat[rows].reshape(NRB, 128).T  # [128, 8]; col rb = shift for that block
        bias_c = np.concatenate([-mh, mh - np.float32(127.0 * LN2)], axis=1).astype(
            np.float32
        )  # [128, 16]: exp bias | lse add-back (frexp's -127*ln2 folded in)
        G_c = np.ascontiguousarray(G[MPC * c : MPC * (c + 1), :].astype(np.float16))
        mrows = np.arange(MPC * c, MPC * (c + 1))
        B_c = np.ascontiguousarray(
            Baug[mrows].reshape(NMC, 128, JA).transpose(1, 0, 2)
        )  # [128, 4, 130]
        in_maps.append(
            {
                "gT": gT_c,
                "G": G_c,
                "C": C_packed,
                "Baug": B_c,
                "bias": bias_c,
                "mask": mask,
            }
        )

    import os

    trace = bool(int(os.environ.get("BASS_KERNEL_TRACE", "0")))
    res = run_bass_kernel_spmd(
        _get_nc(), in_maps, core_ids=list(range(NCORES)), trace=trace
    )
    global LAST_RESULTS
    LAST_RESULTS = res

    lse_sum = 0.0
    graph_sum = 0.0
    for c in range(NCORES):
        o = res.results[c]["out"].astype(np.float64)
        lse_sum += float(o[:, 0].sum())
        graph_sum += float(o[:, 1].sum())

    pos_total = 4.0 * float(
        np.sum(h_i.astype(np.float64) * h_j.astype(np.float64))
    )
    loss = (lse_sum - pos_total) / N + graph_sum / (float(B) * float(K))
    return np.float32(loss)
